# revision 21
# baseline (speedup 1.0000x reference)
"""Trainium2 Bass kernel for a post-LN transformer decoder layer.

Sharding: 8 cores = 4 batches x 2 query-row halves. Each core computes its
batch's full K/V projections (both attention blocks) but only its 512 query
rows; causal balance via interleaved query tile sets {0,3,4,7} / {1,2,5,6}.
No collectives; the host gathers per-core outputs.

On-chip layout: activations are feature-major ("transposed": [d on partitions,
seq on free]) everywhere, so every matmul is out = lhsT.T @ rhs with weights as
lhsT and activations as rhs, and no on-chip transposes are needed. Attention
scores are computed transposed [k, q]; softmax denominators come from a ones
column appended to V (PE accumulates sum of exp along k); the padding mask is
applied as a per-partition bias inside the ScalarE exp; the causal mask is a
0/1 multiply on GpSimd. LayerNorm stats (sums over d = partitions) are
computed with ones-vector matmuls on the PE. Matmuls run in bf16 with fp32
PSUM accumulation; residuals and normalization are fp32.

Scheduling notes: CA K/V projections are emitted before SA attention so the
PE has dense filler work while attention waits on the ScalarE exp chain;
O-proj and FFN matmul loops are kc-outer over 4-chunk m-groups so they start
as soon as the first rhs chunk is ready; score matmuls for a head pair are
issued to distinct PE row groups (tile_position) so they run concurrently.
"""

import sys

for _p in ("/opt/trn_rl_repo", "/opt/pypackages"):
    if _p not in sys.path:
        sys.path.insert(0, _p)

import numpy as np
import ml_dtypes

import concourse.bass as bass
import concourse.tile as tile
from concourse import bacc, mybir

BF16 = ml_dtypes.bfloat16
F32 = mybir.dt.float32
BF = mybir.dt.bfloat16

P = 128
D = 1024
S = 1024
H = 16
DH = 64
DFF = 4096
NKC = D // P           # 8 contraction chunks of 128
Q = 512                # query rows per core
NQT = Q // P           # 4 query tiles per core
NKMAX = [2, 4, 6, 8]   # per local q-tile: max key tiles over the two halves
TILES_EVEN = [0, 3, 4, 7]
TILES_ODD = [1, 2, 5, 6]
SCALE = 1.0 / 8.0      # 1/sqrt(DH)
EPS = 1e-6

WSTREAM_ORDER = [
    "w_sa_q", "w_sa_k", "w_sa_v", "w_ca_k", "w_ca_v", "w_sa_o",
    "w_ca_q", "w_ca_o",
    "w_ff1_0", "w_ff2_0", "w_ff1_1", "w_ff2_1",
    "w_ff1_2", "w_ff2_2", "w_ff1_3", "w_ff2_3",
]


def build_program():
    nc = bacc.Bacc(None, target_bir_lowering=False, debug=False)

    def inp(name, shape, dtype):
        return nc.declare_dram_parameter(name, list(shape), dtype, isOutput=False)[:]

    xT = inp("xT", (P, NKC, S), BF)
    xqT = inp("xqT", (P, NKC, Q), BF)
    xqf = inp("xqf", (P, NKC, Q), F32)
    encT = inp("encT", (P, NKC, S), BF)
    w_dram = {n: inp(n, (P, NKC, 1024), BF) for n in WSTREAM_ORDER}
    binmask_d = inp("binmask", (P, NQT, 2, P), BF)
    crossbias_d = inp("crossbias", (P, NKC), F32)
    lng = {}
    for i in (1, 2, 3):
        lng[f"ln{i}g"] = inp(f"ln{i}g", (P, NKC), F32)
        lng[f"ln{i}b"] = inp(f"ln{i}b", (P, NKC), F32)
    ffb1_d = inp("ffb1", (P, 32), F32)
    ffb2_d = inp("ffb2", (P, NKC), F32)

    outT = nc.declare_dram_parameter("outT", [P, NKC, Q], F32, isOutput=True)[:]
    ln1f_dram = nc.dram_tensor("ln1f_scratch", [P, NKC, Q], F32)[:]

    with tile.TileContext(nc) as tc:
        with (
            tc.tile_pool(name="singles", bufs=1) as singles,
            tc.tile_pool(name="wstream", bufs=2) as wpool,
            tc.tile_pool(name="work", bufs=1) as work,
            tc.tile_pool(name="pt", bufs=3) as ptpool,
            tc.tile_pool(name="chunk", bufs=2) as chunk,
            tc.tile_pool(name="small", bufs=2) as small,
            # PSUM: 8 banks = sc(2x2-bank score pairs) + mm(2) + acc(2)
            tc.tile_pool(name="sc", bufs=2, space="PSUM") as pssc,
            tc.tile_pool(name="mm", bufs=2, space="PSUM") as psmm,
            tc.tile_pool(name="pacc", bufs=2, space="PSUM") as psacc,
        ):
            def wtile(name):
                t = wpool.tile([P, NKC, 1024], BF, tag="w", name=name)
                nc.sync.dma_start(t, w_dram[name])
                return t

            # --- resident small tensors ---------------------------------
            # first-consumed tensors first: the SA Q projection only needs
            # xqT + w_sa_q, so their DMA descriptors go to the queues ahead
            # of the bulky loads below
            xqT_sb = singles.tile([P, NKC, Q], BF, tag="xqT")
            nc.sync.dma_start(xqT_sb, xqT)
            w_saq_sb = wtile("w_sa_q")
            xT_sb = singles.tile([P, NKC, S], BF, tag="xT")
            nc.sync.dma_start(xT_sb, xT)
            w_sak_sb = wtile("w_sa_k")
            encT_sb = singles.tile([P, NKC, S], BF, tag="encT")
            nc.sync.dma_start(encT_sb, encT)
            binmask_sb = singles.tile([P, NQT, 2, P], BF, tag="binmask")
            nc.sync.dma_start(binmask_sb, binmask_d)
            crossbias_sb = singles.tile([P, NKC], F32, tag="crossbias")
            nc.sync.dma_start(crossbias_sb, crossbias_d)
            lng_sb = {}
            for k, ap in lng.items():
                t = singles.tile([P, NKC], F32, tag=k, name=k)
                nc.sync.dma_start(t, ap)
                lng_sb[k] = t
            ffb1_sb = singles.tile([P, 32], F32, tag="ffb1")
            nc.sync.dma_start(ffb1_sb, ffb1_d)
            ffb2_sb = singles.tile([P, NKC], F32, tag="ffb2")
            nc.sync.dma_start(ffb2_sb, ffb2_d)

            ones_col = singles.tile([P, 1], BF, tag="ones_col")
            nc.vector.memset(ones_col, 1.0)
            ones_row = singles.tile([1, P], BF, tag="ones_row")
            nc.vector.memset(ones_row, 1.0)
            zbias = singles.tile([P, 1], F32, tag="zbias")
            nc.vector.memset(zbias, 0.0)
            eps_t = singles.tile([1, 1], F32, tag="eps")
            nc.vector.memset(eps_t, EPS)

            # --- helpers -------------------------------------------------
            def proj_T(out_sb, w_sb, rhs_sb, s_len):
                """out_sb[:, m, s] (bf16) = (W.T @ rhs) in transposed layout."""
                for m in range(8):
                    for sc in range(s_len // 512):
                        ps = psmm.tile([P, 512], F32, tag="mm", name="ps_proj")
                        for kc in range(NKC):
                            nc.tensor.matmul(
                                ps,
                                w_sb[:, kc, m * P:(m + 1) * P],
                                rhs_sb[:, kc, sc * 512:(sc + 1) * 512],
                                start=(kc == 0),
                                stop=(kc == NKC - 1),
                            )
                        nc.vector.tensor_copy(
                            out_sb[:, m, sc * 512:(sc + 1) * 512], ps)

            def vproj(v_sb, srcT_sb, w_sb):
                """v_sb[p, sc, h, 0:64] = V rows; col 64 stays 1.0 (denom)."""
                nc.vector.memset(v_sb[:, :, :, 64:65], 1.0)
                for sc in range(8):
                    for nh in range(2):
                        ps = psmm.tile([P, 512], F32, tag="mm", name="ps_v")
                        for kc in range(NKC):
                            nc.tensor.matmul(
                                ps,
                                srcT_sb[:, kc, sc * P:(sc + 1) * P],
                                w_sb[:, kc, nh * 512:(nh + 1) * 512],
                                start=(kc == 0),
                                stop=(kc == NKC - 1),
                            )
                        nc.vector.tensor_copy(
                            v_sb[:, sc, nh * 8:(nh + 1) * 8, 0:64],
                            ps.rearrange("p (h d) -> p h d", d=DH),
                        )

            def attn_scores(hc, kt, nk, use_binmask, bias_sb):
                """Head-pair scores -> one fused exp -> causal mask."""
                lmin = next(l for l in range(NQT) if nk[l] > kt)
                c = lmin * P
                ps_s = pssc.tile([P, 2, 512], F32, tag="sc", name="ps_s")
                for j, hp in enumerate((0, DH)):
                    nc.tensor.matmul(
                        ps_s[:, j, c:],
                        kt_ref[0][hp:hp + DH, hc, kt * P:(kt + 1) * P],
                        qt_ref[0][hp:hp + DH, hc, c:],
                        start=True, stop=True,
                        tile_position=(hp, 0),
                    )
                pt = ptpool.tile([P, 2, 512], BF, tag="pt", name="pt")
                bias = bias_sb[:, kt:kt + 1] if bias_sb is not None else zbias
                nc.scalar.activation(
                    pt[:, :, c:], ps_s[:, :, c:],
                    mybir.ActivationFunctionType.Exp,
                    bias=bias, scale=SCALE,
                )
                if use_binmask:
                    for l in range(lmin, NQT):
                        j = kt - (nk[l] - 2)
                        if j >= 0:
                            nc.gpsimd.tensor_mul(
                                pt[:, :, l * P:(l + 1) * P],
                                pt[:, :, l * P:(l + 1) * P],
                                binmask_sb[:, l, j:j + 1, :].to_broadcast(
                                    (P, 2, P)),
                            )
                return pt, c

            def attn_av(hp, hc, kt, pt, c, ps_o, nkt):
                h = hc * 2 + (hp // DH)
                nc.tensor.matmul(
                    ps_o[:65, c:],
                    v_ref[0][:, kt, h, :],
                    pt[:, hp // DH, c:],
                    start=(kt == 0),
                    stop=(kt == nkt - 1),
                    skip_group_check=True,
                )

            def attn_norm(hp, hc, ps_o, o_sb):
                dbf = small.tile([1, 512], BF, tag="dbf", bufs=2, name="dbf")
                nc.vector.tensor_copy(dbf, ps_o[64:65, :])
                ps_db = psmm.tile([P, 512], F32, tag="mm", name="ps_db")
                nc.tensor.matmul(ps_db[:DH, :], ones_row[0:1, 0:DH], dbf,
                                 start=True, stop=True)
                recip = work.tile([DH, 512], F32, tag="recip", bufs=1,
                                  name="recip")
                nc.vector.reciprocal_approx_fast(recip, ps_db[:DH, :])
                nc.vector.tensor_mul(
                    o_sb[hp:hp + DH, hc, :], ps_o[0:DH, :], recip)

            def attention(qt_sb, kt_sb, v_sb, nk, use_binmask, bias_sb, o_sb):
                qt_ref[0], kt_ref[0], v_ref[0] = qt_sb, kt_sb, v_sb
                nkt = max(nk)
                for hc in range(8):
                    ps_o0 = psacc.tile([P, 512], F32, tag="acc", name="ps_o0")
                    ps_o1 = psacc.tile([P, 512], F32, tag="acc", name="ps_o1")
                    # scores/exp run one kt ahead of the AV consumers so the
                    # in-order PE queue never stalls on the exp roundtrip
                    prev = None
                    for kt in range(nkt):
                        ptp, c = attn_scores(hc, kt, nk, use_binmask, bias_sb)
                        if prev is not None:
                            pkt, pp, pc = prev
                            attn_av(0, hc, pkt, pp, pc, ps_o0, nkt)
                            attn_av(DH, hc, pkt, pp, pc, ps_o1, nkt)
                        prev = (kt, ptp, c)
                    pkt, pp, pc = prev
                    attn_av(0, hc, pkt, pp, pc, ps_o0, nkt)
                    attn_av(DH, hc, pkt, pp, pc, ps_o1, nkt)
                    attn_norm(0, hc, ps_o0, o_sb)
                    attn_norm(DH, hc, ps_o1, o_sb)

            qt_ref, kt_ref, v_ref = [None], [None], [None]

            def out_proj(w_sb, o_sb, resid_fn):
                """yT = W.T @ o + residual; resid_fn(m, ps) consumes psum."""
                for m in range(NKC):
                    ps = psmm.tile([P, 512], F32, tag="mm", name="ps_op")
                    for kc in range(NKC):
                        nc.tensor.matmul(
                            ps,
                            w_sb[:, kc, m * P:(m + 1) * P],
                            o_sb[:, kc, :],
                            start=(kc == 0), stop=(kc == NKC - 1),
                        )
                    resid_fn(m, ps)

            def layer_norm(r_sb, gk, bk, out_bf, f32_dst_fn, f32_post_fn=None):
                """r_sb [P, NKC, 512] f32 -> out_bf (opt) + f32 chunks."""
                ps_s1 = psmm.tile([1, 512], F32, tag="mm", name="ps_s1")
                ps_s2 = psmm.tile([1, 512], F32, tag="mm", name="ps_s2")
                for m in range(NKC):
                    rbf = chunk.tile([P, 512], BF, tag="rbf", name="rbf")
                    nc.vector.tensor_copy(rbf, r_sb[:, m, :])
                    sq = chunk.tile([P, 512], BF, tag="sq", name="sq")
                    nc.scalar.activation(sq, r_sb[:, m, :],
                                         mybir.ActivationFunctionType.Square,
                                         bias=zbias)
                    nc.tensor.matmul(ps_s1, ones_col, rbf,
                                     start=(m == 0), stop=(m == NKC - 1))
                    nc.tensor.matmul(ps_s2, ones_col, sq,
                                     start=(m == 0), stop=(m == NKC - 1))
                mean = small.tile([1, 512], F32, tag="sm1", bufs=1, name="mean")
                nc.vector.tensor_scalar_mul(mean, ps_s1, 1.0 / D)
                var = small.tile([1, 512], F32, tag="sm2", bufs=1, name="var")
                nc.vector.tensor_scalar_mul(var, ps_s2, 1.0 / D)
                scr = small.tile([1, 512], F32, tag="sm3", bufs=1, name="scr")
                nc.vector.tensor_mul(scr, mean, mean)
                nc.vector.tensor_sub(var, var, scr)
                nc.scalar.activation(var, var,
                                     mybir.ActivationFunctionType.Sqrt,
                                     bias=eps_t)
                # broadcast mean first: its bcast matmul overlaps the
                # sqrt/recip chain that produces rstd
                mean_bf = small.tile([1, 512], BF, tag="smb", bufs=2,
                                     name="mean_bf")
                nc.vector.tensor_copy(mean_bf, mean)
                ps_mb = psacc.tile([P, 512], F32, tag="acc", name="ps_mb")
                nc.tensor.matmul(ps_mb, ones_row[0:1, :], mean_bf,
                                 start=True, stop=True)
                rstd = scr  # in-place reuse of the scratch tile
                nc.vector.reciprocal_approx_fast(rstd, var)
                rstd_bf = small.tile([1, 512], BF, tag="smb", bufs=2,
                                     name="rstd_bf")
                nc.vector.tensor_copy(rstd_bf, rstd)
                ps_rb = psacc.tile([P, 512], F32, tag="acc", name="ps_rb")
                nc.tensor.matmul(ps_rb, ones_row[0:1, :], rstd_bf,
                                 start=True, stop=True)
                g_sb = lng_sb[gk]
                b_sb = lng_sb[bk]
                for m in range(NKC):
                    t = chunk.tile([P, 512], F32, tag="lntmp", name="lnt")
                    nc.vector.tensor_sub(t, r_sb[:, m, :], ps_mb)
                    nc.vector.tensor_mul(t, t, ps_rb)
                    fchunk = f32_dst_fn(m)
                    nc.vector.tensor_scalar(
                        fchunk, t, g_sb[:, m:m + 1], b_sb[:, m:m + 1],
                        op0=mybir.AluOpType.mult, op1=mybir.AluOpType.add)
                    if out_bf is not None:
                        nc.vector.tensor_copy(out_bf[:, m, :], fchunk)
                    if f32_post_fn is not None:
                        f32_post_fn(m, fchunk)

            # =============== self-attention block =======================
            qt_sb = work.tile([P, NKC, Q], BF, tag="qt", name="qt_sa")
            proj_T(qt_sb, w_saq_sb, xqT_sb, Q)
            kt_sb = work.tile([P, NKC, S], BF, tag="kt", name="kt_sa")
            proj_T(kt_sb, w_sak_sb, xT_sb, S)
            w = wtile("w_sa_v")
            v_sb = work.tile([P, 8, H, 65], BF, tag="v", name="v_sa")
            vproj(v_sb, xT_sb, w)

            # CA K/V projections emitted early: dense PE filler work that
            # overlaps the latency-bound SA attention chain.
            w = wtile("w_ca_k")
            kt2_sb = work.tile([P, NKC, S], BF, tag="kt2", name="kt_ca")
            proj_T(kt2_sb, w, encT_sb, S)
            w = wtile("w_ca_v")
            v2_sb = work.tile([P, 8, H, 65], BF, tag="v2", name="v_ca")
            vproj(v2_sb, encT_sb, w)

            o_sb = work.tile([P, NKC, Q], BF, tag="ot", name="ot_sa")
            attention(qt_sb, kt_sb, v_sb, NKMAX, True, None, o_sb)

            w = wtile("w_sa_o")
            rT = work.tile([P, NKC, Q], F32, tag="rT", name="r1")

            def sa_resid(m, ps):
                xq_ch = chunk.tile([P, 512], F32, tag="stream", bufs=3, name="xq_ch")
                nc.sync.dma_start(xq_ch, xqf[:, m, :])
                nc.vector.tensor_add(rT[:, m, :], ps, xq_ch)

            out_proj(w, o_sb, sa_resid)

            # shares the xT slot (released after SA K/V projections)
            ln1bf = singles.tile([P, NKC, Q], BF, tag="xT", name="ln1bf")
            layer_norm(
                rT, "ln1g", "ln1b", ln1bf,
                lambda m: chunk.tile([P, 512], F32, tag="stream", bufs=3,
                                     name="ln1f_ch"),
                lambda m, fc: nc.sync.dma_start(ln1f_dram[:, m, :], fc))

            # =============== cross-attention block ======================
            w = wtile("w_ca_q")
            qt2_sb = work.tile([P, NKC, Q], BF, tag="qt", name="qt_ca")
            proj_T(qt2_sb, w, ln1bf, Q)

            o2_sb = work.tile([P, NKC, Q], BF, tag="ot", name="ot_ca")
            attention(qt2_sb, kt2_sb, v2_sb, [8] * NQT, False, crossbias_sb,
                      o2_sb)

            w = wtile("w_ca_o")
            r2 = work.tile([P, NKC, Q], F32, tag="rT", name="r2")

            def ca_resid(m, ps):
                l1_ch = chunk.tile([P, 512], F32, tag="stream", bufs=3, name="l1_ch")
                nc.sync.dma_start(l1_ch, ln1f_dram[:, m, :])
                nc.vector.tensor_add(r2[:, m, :], ps, l1_ch)

            out_proj(w, o2_sb, ca_resid)

            ln2bf = singles.tile([P, NKC, Q], BF, tag="xT", name="ln2bf")
            # reuses the SA KT slot (released after SA attention)
            y_acc = work.tile([P, NKC, Q], F32, tag="kt", name="y_acc")
            layer_norm(r2, "ln2g", "ln2b", ln2bf,
                       lambda m: y_acc[:, m, :])

            # =============== FFN ========================================
            for qi in range(4):
                w1 = wtile(f"w_ff1_{qi}")
                # shares the encT slot (released after CA K/V projections)
                hT = singles.tile([P, NKC, Q], BF, tag="encT", name=f"hT_{qi}")
                for mc in range(NKC):
                    ps = psmm.tile([P, 512], F32, tag="mm", name="ps_f1")
                    for kc in range(NKC):
                        nc.tensor.matmul(
                            ps,
                            w1[:, kc, mc * P:(mc + 1) * P],
                            ln2bf[:, kc, :],
                            start=(kc == 0), stop=(kc == NKC - 1))
                    nc.scalar.activation(
                        hT[:, mc, :], ps,
                        mybir.ActivationFunctionType.Relu,
                        bias=ffb1_sb[:, qi * 8 + mc:qi * 8 + mc + 1])
                w2 = wtile(f"w_ff2_{qi}")
                for mc in range(NKC):
                    ps = psmm.tile([P, 512], F32, tag="mm", name="ps_f2")
                    for kc in range(NKC):
                        nc.tensor.matmul(
                            ps,
                            w2[:, kc, mc * P:(mc + 1) * P],
                            hT[:, kc, :],
                            start=(kc == 0), stop=(kc == NKC - 1))
                    nc.vector.tensor_add(y_acc[:, mc, :], y_acc[:, mc, :],
                                         ps)

            for m in range(NKC):
                nc.vector.tensor_scalar_add(y_acc[:, m, :], y_acc[:, m, :],
                                            ffb2_sb[:, m:m + 1])

            # =============== final LN + output ==========================
            layer_norm(
                y_acc, "ln3g", "ln3b", None,
                lambda m: chunk.tile([P, 512], F32, tag="stream", bufs=3,
                                     name="out_ch"),
                lambda m, fc: nc.sync.dma_start(outT[:, m, :], fc))

    nc.finalize()
    return nc


# ---------------------------------------------------------------------------
# host side
# ---------------------------------------------------------------------------

def _wl(w):
    """[K, N] -> [128, K/128, N] bf16 (K on partitions, chunked)."""
    k, n = w.shape
    return np.ascontiguousarray(
        w.reshape(k // P, P, n).transpose(1, 0, 2)).astype(BF16)


def _tl(a):
    """[S, D] activation -> transposed layout [128, D/128, S]."""
    t = a.T  # [D, S]
    return np.ascontiguousarray(
        t.reshape(D // P, P, t.shape[1]).transpose(1, 0, 2))


def _vc(v, n):
    """[n*128] vector -> [128, n] (chunk layout)."""
    return np.ascontiguousarray(v.reshape(n, P).T)


def _binmask(tiles):
    m = np.zeros((P, NQT, 2, P), np.float32)
    tri = (np.arange(P)[:, None] <= np.arange(P)[None, :]).astype(np.float32)
    for l in range(NQT):
        g = tiles[l]
        for j in range(2):
            kt = NKMAX[l] - 2 + j
            if kt < g:
                m[:, l, j, :] = 1.0
            elif kt == g:
                m[:, l, j, :] = tri
            else:
                m[:, l, j, :] = 0.0
    return m.astype(BF16)


_NC_CACHE = None


def _get_nc():
    global _NC_CACHE
    if _NC_CACHE is None:
        _NC_CACHE = build_program()
    return _NC_CACHE


def make_in_maps(inputs):
    x = np.asarray(inputs["decoder_inputs"], np.float32)
    enc = np.asarray(inputs["encoder_output"], np.float32)
    pad = np.asarray(inputs["padding_mask"], np.float32)

    shared = {
        "w_sa_q": _wl(np.asarray(inputs["sa_wq"], np.float32)),
        "w_sa_k": _wl(np.asarray(inputs["sa_wk"], np.float32)),
        "w_sa_v": _wl(np.asarray(inputs["sa_wv"], np.float32)),
        "w_sa_o": _wl(np.asarray(inputs["sa_wo"], np.float32)),
        "w_ca_q": _wl(np.asarray(inputs["ca_wq"], np.float32)),
        "w_ca_k": _wl(np.asarray(inputs["ca_wk"], np.float32)),
        "w_ca_v": _wl(np.asarray(inputs["ca_wv"], np.float32)),
        "w_ca_o": _wl(np.asarray(inputs["ca_wo"], np.float32)),
    }
    ffw1 = np.asarray(inputs["ff_w1"], np.float32)
    ffw2 = np.asarray(inputs["ff_w2"], np.float32)
    for qi in range(4):
        shared[f"w_ff1_{qi}"] = _wl(ffw1[:, qi * 1024:(qi + 1) * 1024])
        shared[f"w_ff2_{qi}"] = _wl(ffw2[qi * 1024:(qi + 1) * 1024, :])
    for i in (1, 2, 3):
        shared[f"ln{i}g"] = _vc(np.asarray(inputs[f"ln{i}_g"], np.float32), NKC)
        shared[f"ln{i}b"] = _vc(np.asarray(inputs[f"ln{i}_b"], np.float32), NKC)
    shared["ffb1"] = _vc(np.asarray(inputs["ff_b1"], np.float32), 32)
    shared["ffb2"] = _vc(np.asarray(inputs["ff_b2"], np.float32), NKC)

    bm_even = _binmask(TILES_EVEN)
    bm_odd = _binmask(TILES_ODD)

    in_maps = []
    for c in range(8):
        b = c // 2
        tiles = TILES_EVEN if c % 2 == 0 else TILES_ODD
        qrows = np.concatenate(
            [np.arange(t * P, (t + 1) * P) for t in tiles])
        xb = x[b]
        m = dict(shared)
        m["xT"] = _tl(xb).astype(BF16)
        xq = xb[qrows]
        xqt = _tl(xq)
        m["xqT"] = xqt.astype(BF16)
        m["xqf"] = xqt.astype(np.float32)
        m["encT"] = _tl(enc[b]).astype(BF16)
        m["binmask"] = bm_even if c % 2 == 0 else bm_odd
        m["crossbias"] = _vc(pad[b, 0, 0, :] * np.float32(-1e9), NKC)
        in_maps.append(m)
    return in_maps


def assemble_output(results):
    out = np.empty((4, S, D), np.float32)
    for c in range(8):
        b = c // 2
        tiles = TILES_EVEN if c % 2 == 0 else TILES_ODD
        ot = np.asarray(results[c]["outT"], np.float32).reshape(P, NKC, Q)
        full_t = ot.transpose(1, 0, 2).reshape(D, Q)     # [d, qlocal]
        for l, t in enumerate(tiles):
            out[b, t * P:(t + 1) * P, :] = full_t[:, l * P:(l + 1) * P].T
    return out


def kernel(**inputs):
    from concourse.bass_utils import run_bass_kernel_spmd

    nc = _get_nc()
    in_maps = make_in_maps(inputs)
    res = run_bass_kernel_spmd(nc, in_maps, core_ids=list(range(8)))
    return assemble_output(res.results)


# revision 22
# speedup vs baseline: 1.0200x; 1.0200x over previous
"""Trainium2 Bass kernel for a post-LN transformer decoder layer.

Sharding: 8 cores = 4 batches x 2 query-row halves. Each core computes its
batch's full K/V projections (both attention blocks) but only its 512 query
rows; causal balance via interleaved query tile sets {0,3,4,7} / {1,2,5,6}.
No collectives; the host gathers per-core outputs.

On-chip layout: activations are feature-major ("transposed": [d on partitions,
seq on free]) everywhere, so every matmul is out = lhsT.T @ rhs with weights as
lhsT and activations as rhs, and no on-chip transposes are needed. Attention
scores are computed transposed [k, q]; softmax denominators come from a ones
column appended to V (PE accumulates sum of exp along k); the padding mask is
applied as a per-partition bias inside the ScalarE exp; the causal mask is a
0/1 multiply on GpSimd. LayerNorm stats (sums over d = partitions) are
computed with ones-vector matmuls on the PE. Matmuls run in bf16 with fp32
PSUM accumulation; residuals and normalization are fp32.

Scheduling notes: CA K/V projections are emitted before SA attention so the
PE has dense filler work while attention waits on the ScalarE exp chain;
O-proj and FFN matmul loops are kc-outer over 4-chunk m-groups so they start
as soon as the first rhs chunk is ready; score matmuls for a head pair are
issued to distinct PE row groups (tile_position) so they run concurrently.
"""

import sys

for _p in ("/opt/trn_rl_repo", "/opt/pypackages"):
    if _p not in sys.path:
        sys.path.insert(0, _p)

import numpy as np
import ml_dtypes

import concourse.bass as bass
import concourse.tile as tile
from concourse import bacc, mybir

BF16 = ml_dtypes.bfloat16
F32 = mybir.dt.float32
BF = mybir.dt.bfloat16

P = 128
D = 1024
S = 1024
H = 16
DH = 64
DFF = 4096
NKC = D // P           # 8 contraction chunks of 128
Q = 512                # query rows per core
NQT = Q // P           # 4 query tiles per core
NKMAX = [2, 4, 6, 8]   # per local q-tile: max key tiles over the two halves
TILES_EVEN = [0, 3, 4, 7]
TILES_ODD = [1, 2, 5, 6]
SCALE = 1.0 / 8.0      # 1/sqrt(DH)
EPS = 1e-6

WSTREAM_ORDER = [
    "w_sa_q", "w_sa_k", "w_sa_v", "w_ca_k", "w_ca_v", "w_sa_o",
    "w_ca_q", "w_ca_o",
    "w_ff1_0", "w_ff2_0", "w_ff1_1", "w_ff2_1",
    "w_ff1_2", "w_ff2_2", "w_ff1_3", "w_ff2_3",
]


def build_program():
    nc = bacc.Bacc(None, target_bir_lowering=False, debug=False)

    def inp(name, shape, dtype):
        return nc.declare_dram_parameter(name, list(shape), dtype, isOutput=False)[:]

    xT = inp("xT", (P, NKC, S), BF)
    xqT = inp("xqT", (P, NKC, Q), BF)
    xqf = inp("xqf", (P, NKC, Q), F32)
    encT = inp("encT", (P, NKC, S), BF)
    w_dram = {n: inp(n, (P, NKC, 1024), BF) for n in WSTREAM_ORDER}
    binmask_d = inp("binmask", (P, NQT, 2, P), BF)
    crossbias_d = inp("crossbias", (P, NKC), F32)
    lng = {}
    for i in (1, 2, 3):
        lng[f"ln{i}g"] = inp(f"ln{i}g", (P, NKC), F32)
        lng[f"ln{i}b"] = inp(f"ln{i}b", (P, NKC), F32)
    ffb1_d = inp("ffb1", (P, 32), F32)
    ffb2_d = inp("ffb2", (P, NKC), F32)

    outT = nc.declare_dram_parameter("outT", [P, NKC, Q], F32, isOutput=True)[:]
    ln1f_dram = nc.dram_tensor("ln1f_scratch", [P, NKC, Q], F32)[:]

    with tile.TileContext(nc) as tc:
        with (
            tc.tile_pool(name="singles", bufs=1) as singles,
            tc.tile_pool(name="wstream", bufs=2) as wpool,
            tc.tile_pool(name="work", bufs=1) as work,
            tc.tile_pool(name="pt", bufs=5) as ptpool,
            tc.tile_pool(name="chunk", bufs=2) as chunk,
            tc.tile_pool(name="small", bufs=2) as small,
            # PSUM: 8 banks = sc(2x2-bank score pairs) + mm(2) + acc(2)
            tc.tile_pool(name="sc", bufs=2, space="PSUM") as pssc,
            tc.tile_pool(name="mm", bufs=2, space="PSUM") as psmm,
            tc.tile_pool(name="pacc", bufs=2, space="PSUM") as psacc,
        ):
            def wtile(name):
                t = wpool.tile([P, NKC, 1024], BF, tag="w", name=name)
                nc.sync.dma_start(t, w_dram[name])
                return t

            # --- resident small tensors ---------------------------------
            # first-consumed tensors first: the SA Q projection only needs
            # xqT + w_sa_q, so their DMA descriptors go to the queues ahead
            # of the bulky loads below
            xqT_sb = singles.tile([P, NKC, Q], BF, tag="xqT")
            nc.sync.dma_start(xqT_sb, xqT)
            w_saq_sb = wtile("w_sa_q")
            xT_sb = singles.tile([P, NKC, S], BF, tag="xT")
            nc.sync.dma_start(xT_sb, xT)
            w_sak_sb = wtile("w_sa_k")
            encT_sb = singles.tile([P, NKC, S], BF, tag="encT")
            nc.sync.dma_start(encT_sb, encT)
            binmask_sb = singles.tile([P, NQT, 2, P], BF, tag="binmask")
            nc.sync.dma_start(binmask_sb, binmask_d)
            crossbias_sb = singles.tile([P, NKC], F32, tag="crossbias")
            nc.sync.dma_start(crossbias_sb, crossbias_d)
            lng_sb = {}
            for k, ap in lng.items():
                t = singles.tile([P, NKC], F32, tag=k, name=k)
                nc.sync.dma_start(t, ap)
                lng_sb[k] = t
            ffb1_sb = singles.tile([P, 32], F32, tag="ffb1")
            nc.sync.dma_start(ffb1_sb, ffb1_d)
            ffb2_sb = singles.tile([P, NKC], F32, tag="ffb2")
            nc.sync.dma_start(ffb2_sb, ffb2_d)

            ones_col = singles.tile([P, 1], BF, tag="ones_col")
            nc.vector.memset(ones_col, 1.0)
            ones_row = singles.tile([1, P], BF, tag="ones_row")
            nc.vector.memset(ones_row, 1.0)
            zbias = singles.tile([P, 1], F32, tag="zbias")
            nc.vector.memset(zbias, 0.0)
            eps_t = singles.tile([1, 1], F32, tag="eps")
            nc.vector.memset(eps_t, EPS)

            # --- helpers -------------------------------------------------
            def proj_T(out_sb, w_sb, rhs_sb, s_len):
                """out_sb[:, m, s] (bf16) = (W.T @ rhs) in transposed layout."""
                for m in range(8):
                    for sc in range(s_len // 512):
                        ps = psmm.tile([P, 512], F32, tag="mm", name="ps_proj")
                        for kc in range(NKC):
                            nc.tensor.matmul(
                                ps,
                                w_sb[:, kc, m * P:(m + 1) * P],
                                rhs_sb[:, kc, sc * 512:(sc + 1) * 512],
                                start=(kc == 0),
                                stop=(kc == NKC - 1),
                            )
                        nc.vector.tensor_copy(
                            out_sb[:, m, sc * 512:(sc + 1) * 512], ps)

            def vproj(v_sb, srcT_sb, w_sb):
                """v_sb[p, sc, h, 0:64] = V rows; col 64 stays 1.0 (denom)."""
                nc.vector.memset(v_sb[:, :, :, 64:65], 1.0)
                for sc in range(8):
                    for nh in range(2):
                        ps = psmm.tile([P, 512], F32, tag="mm", name="ps_v")
                        for kc in range(NKC):
                            nc.tensor.matmul(
                                ps,
                                srcT_sb[:, kc, sc * P:(sc + 1) * P],
                                w_sb[:, kc, nh * 512:(nh + 1) * 512],
                                start=(kc == 0),
                                stop=(kc == NKC - 1),
                            )
                        nc.vector.tensor_copy(
                            v_sb[:, sc, nh * 8:(nh + 1) * 8, 0:64],
                            ps.rearrange("p (h d) -> p h d", d=DH),
                        )

            def attn_scores(hc, kt, nk, use_binmask, bias_sb):
                """Head-pair scores -> one fused exp -> causal mask."""
                lmin = next(l for l in range(NQT) if nk[l] > kt)
                c = lmin * P
                ps_s = pssc.tile([P, 2, 512], F32, tag="sc", name="ps_s")
                for j, hp in enumerate((0, DH)):
                    nc.tensor.matmul(
                        ps_s[:, j, c:],
                        kt_ref[0][hp:hp + DH, hc, kt * P:(kt + 1) * P],
                        qt_ref[0][hp:hp + DH, hc, c:],
                        start=True, stop=True,
                        tile_position=(hp, 0),
                    )
                pt = ptpool.tile([P, 2, 512], BF, tag="pt", name="pt")
                bias = bias_sb[:, kt:kt + 1] if bias_sb is not None else zbias
                nc.scalar.activation(
                    pt[:, :, c:], ps_s[:, :, c:],
                    mybir.ActivationFunctionType.Exp,
                    bias=bias, scale=SCALE,
                )
                if use_binmask:
                    for l in range(lmin, NQT):
                        j = kt - (nk[l] - 2)
                        if j >= 0:
                            nc.gpsimd.tensor_mul(
                                pt[:, :, l * P:(l + 1) * P],
                                pt[:, :, l * P:(l + 1) * P],
                                binmask_sb[:, l, j:j + 1, :].to_broadcast(
                                    (P, 2, P)),
                            )
                return pt, c

            def attn_av(hp, hc, kt, pt, c, ps_o, nkt):
                h = hc * 2 + (hp // DH)
                nc.tensor.matmul(
                    ps_o[:65, c:],
                    v_ref[0][:, kt, h, :],
                    pt[:, hp // DH, c:],
                    start=(kt == 0),
                    stop=(kt == nkt - 1),
                    skip_group_check=True,
                )

            def attn_norm(hp, hc, ps_o, o_sb):
                dbf = small.tile([1, 512], BF, tag="dbf", bufs=2, name="dbf")
                nc.vector.tensor_copy(dbf, ps_o[64:65, :])
                ps_db = psmm.tile([P, 512], F32, tag="mm", name="ps_db")
                nc.tensor.matmul(ps_db[:DH, :], ones_row[0:1, 0:DH], dbf,
                                 start=True, stop=True)
                recip = work.tile([DH, 512], F32, tag="recip", bufs=1,
                                  name="recip")
                nc.vector.reciprocal_approx_fast(recip, ps_db[:DH, :])
                nc.vector.tensor_mul(
                    o_sb[hp:hp + DH, hc, :], ps_o[0:DH, :], recip)

            def attention(qt_sb, kt_sb, v_sb, nk, use_binmask, bias_sb, o_sb):
                qt_ref[0], kt_ref[0], v_ref[0] = qt_sb, kt_sb, v_sb
                nkt = max(nk)
                for hc in range(8):
                    ps_o0 = psacc.tile([P, 512], F32, tag="acc", name="ps_o0")
                    ps_o1 = psacc.tile([P, 512], F32, tag="acc", name="ps_o1")
                    # scores/exp run one kt ahead of the AV consumers so the
                    # in-order PE queue never stalls on the exp roundtrip
                    prev = None
                    for kt in range(nkt):
                        ptp, c = attn_scores(hc, kt, nk, use_binmask, bias_sb)
                        if prev is not None:
                            pkt, pp, pc = prev
                            attn_av(0, hc, pkt, pp, pc, ps_o0, nkt)
                            attn_av(DH, hc, pkt, pp, pc, ps_o1, nkt)
                        prev = (kt, ptp, c)
                    pkt, pp, pc = prev
                    attn_av(0, hc, pkt, pp, pc, ps_o0, nkt)
                    attn_av(DH, hc, pkt, pp, pc, ps_o1, nkt)
                    attn_norm(0, hc, ps_o0, o_sb)
                    attn_norm(DH, hc, ps_o1, o_sb)

            qt_ref, kt_ref, v_ref = [None], [None], [None]

            def out_proj(w_sb, o_sb, resid_fn):
                """yT = W.T @ o + residual; resid_fn(m, ps) consumes psum."""
                for m in range(NKC):
                    ps = psmm.tile([P, 512], F32, tag="mm", name="ps_op")
                    for kc in range(NKC):
                        nc.tensor.matmul(
                            ps,
                            w_sb[:, kc, m * P:(m + 1) * P],
                            o_sb[:, kc, :],
                            start=(kc == 0), stop=(kc == NKC - 1),
                        )
                    resid_fn(m, ps)

            def layer_norm(r_sb, gk, bk, out_bf, f32_dst_fn, f32_post_fn=None):
                """r_sb [P, NKC, 512] f32 -> out_bf (opt) + f32 chunks."""
                ps_s1 = psmm.tile([1, 512], F32, tag="mm", name="ps_s1")
                ps_s2 = psmm.tile([1, 512], F32, tag="mm", name="ps_s2")
                for m in range(NKC):
                    rbf = chunk.tile([P, 512], BF, tag="rbf", name="rbf")
                    nc.vector.tensor_copy(rbf, r_sb[:, m, :])
                    sq = chunk.tile([P, 512], BF, tag="sq", name="sq")
                    nc.scalar.activation(sq, r_sb[:, m, :],
                                         mybir.ActivationFunctionType.Square,
                                         bias=zbias)
                    nc.tensor.matmul(ps_s1, ones_col, rbf,
                                     start=(m == 0), stop=(m == NKC - 1))
                    nc.tensor.matmul(ps_s2, ones_col, sq,
                                     start=(m == 0), stop=(m == NKC - 1))
                mean = small.tile([1, 512], F32, tag="sm1", bufs=1, name="mean")
                nc.vector.tensor_scalar_mul(mean, ps_s1, 1.0 / D)
                var = small.tile([1, 512], F32, tag="sm2", bufs=1, name="var")
                nc.vector.tensor_scalar_mul(var, ps_s2, 1.0 / D)
                scr = small.tile([1, 512], F32, tag="sm3", bufs=1, name="scr")
                nc.vector.tensor_mul(scr, mean, mean)
                nc.vector.tensor_sub(var, var, scr)
                nc.scalar.activation(var, var,
                                     mybir.ActivationFunctionType.Sqrt,
                                     bias=eps_t)
                # broadcast mean first: its bcast matmul overlaps the
                # sqrt/recip chain that produces rstd
                mean_bf = small.tile([1, 512], BF, tag="smb", bufs=2,
                                     name="mean_bf")
                nc.vector.tensor_copy(mean_bf, mean)
                ps_mb = psacc.tile([P, 512], F32, tag="acc", name="ps_mb")
                nc.tensor.matmul(ps_mb, ones_row[0:1, :], mean_bf,
                                 start=True, stop=True)
                rstd = scr  # in-place reuse of the scratch tile
                nc.vector.reciprocal_approx_fast(rstd, var)
                rstd_bf = small.tile([1, 512], BF, tag="smb", bufs=2,
                                     name="rstd_bf")
                nc.vector.tensor_copy(rstd_bf, rstd)
                ps_rb = psacc.tile([P, 512], F32, tag="acc", name="ps_rb")
                nc.tensor.matmul(ps_rb, ones_row[0:1, :], rstd_bf,
                                 start=True, stop=True)
                g_sb = lng_sb[gk]
                b_sb = lng_sb[bk]
                for m in range(NKC):
                    t = chunk.tile([P, 512], F32, tag="lntmp", name="lnt")
                    nc.vector.tensor_sub(t, r_sb[:, m, :], ps_mb)
                    nc.vector.tensor_mul(t, t, ps_rb)
                    fchunk = f32_dst_fn(m)
                    nc.vector.tensor_scalar(
                        fchunk, t, g_sb[:, m:m + 1], b_sb[:, m:m + 1],
                        op0=mybir.AluOpType.mult, op1=mybir.AluOpType.add)
                    if out_bf is not None:
                        nc.vector.tensor_copy(out_bf[:, m, :], fchunk)
                    if f32_post_fn is not None:
                        f32_post_fn(m, fchunk)

            # =============== self-attention block =======================
            qt_sb = work.tile([P, NKC, Q], BF, tag="qt", name="qt_sa")
            proj_T(qt_sb, w_saq_sb, xqT_sb, Q)
            kt_sb = work.tile([P, NKC, S], BF, tag="kt", name="kt_sa")
            proj_T(kt_sb, w_sak_sb, xT_sb, S)
            w = wtile("w_sa_v")
            v_sb = work.tile([P, 8, H, 65], BF, tag="v", name="v_sa")
            vproj(v_sb, xT_sb, w)

            # CA K/V projections emitted early: dense PE filler work that
            # overlaps the latency-bound SA attention chain.
            w = wtile("w_ca_k")
            kt2_sb = work.tile([P, NKC, S], BF, tag="kt2", name="kt_ca")
            proj_T(kt2_sb, w, encT_sb, S)
            w = wtile("w_ca_v")
            v2_sb = work.tile([P, 8, H, 65], BF, tag="v2", name="v_ca")
            vproj(v2_sb, encT_sb, w)

            o_sb = work.tile([P, NKC, Q], BF, tag="ot", name="ot_sa")
            attention(qt_sb, kt_sb, v_sb, NKMAX, True, None, o_sb)

            w = wtile("w_sa_o")
            rT = work.tile([P, NKC, Q], F32, tag="rT", name="r1")

            def sa_resid(m, ps):
                xq_ch = chunk.tile([P, 512], F32, tag="stream", bufs=3, name="xq_ch")
                nc.sync.dma_start(xq_ch, xqf[:, m, :])
                nc.vector.tensor_add(rT[:, m, :], ps, xq_ch)

            out_proj(w, o_sb, sa_resid)

            # shares the xT slot (released after SA K/V projections)
            ln1bf = singles.tile([P, NKC, Q], BF, tag="xT", name="ln1bf")
            layer_norm(
                rT, "ln1g", "ln1b", ln1bf,
                lambda m: chunk.tile([P, 512], F32, tag="stream", bufs=3,
                                     name="ln1f_ch"),
                lambda m, fc: nc.sync.dma_start(ln1f_dram[:, m, :], fc))

            # =============== cross-attention block ======================
            w = wtile("w_ca_q")
            qt2_sb = work.tile([P, NKC, Q], BF, tag="qt", name="qt_ca")
            proj_T(qt2_sb, w, ln1bf, Q)

            o2_sb = work.tile([P, NKC, Q], BF, tag="ot", name="ot_ca")
            attention(qt2_sb, kt2_sb, v2_sb, [8] * NQT, False, crossbias_sb,
                      o2_sb)

            w = wtile("w_ca_o")
            r2 = work.tile([P, NKC, Q], F32, tag="rT", name="r2")

            def ca_resid(m, ps):
                l1_ch = chunk.tile([P, 512], F32, tag="stream", bufs=3, name="l1_ch")
                nc.sync.dma_start(l1_ch, ln1f_dram[:, m, :])
                nc.vector.tensor_add(r2[:, m, :], ps, l1_ch)

            out_proj(w, o2_sb, ca_resid)

            ln2bf = singles.tile([P, NKC, Q], BF, tag="xT", name="ln2bf")
            # reuses the SA KT slot (released after SA attention)
            y_acc = work.tile([P, NKC, Q], F32, tag="kt", name="y_acc")
            layer_norm(r2, "ln2g", "ln2b", ln2bf,
                       lambda m: y_acc[:, m, :])

            # =============== FFN ========================================
            for qi in range(4):
                w1 = wtile(f"w_ff1_{qi}")
                # shares the encT slot (released after CA K/V projections)
                hT = singles.tile([P, NKC, Q], BF, tag="encT", name=f"hT_{qi}")
                for mc in range(NKC):
                    ps = psmm.tile([P, 512], F32, tag="mm", name="ps_f1")
                    for kc in range(NKC):
                        nc.tensor.matmul(
                            ps,
                            w1[:, kc, mc * P:(mc + 1) * P],
                            ln2bf[:, kc, :],
                            start=(kc == 0), stop=(kc == NKC - 1))
                    nc.scalar.activation(
                        hT[:, mc, :], ps,
                        mybir.ActivationFunctionType.Relu,
                        bias=ffb1_sb[:, qi * 8 + mc:qi * 8 + mc + 1])
                w2 = wtile(f"w_ff2_{qi}")
                for mc in range(NKC):
                    ps = psmm.tile([P, 512], F32, tag="mm", name="ps_f2")
                    for kc in range(NKC):
                        nc.tensor.matmul(
                            ps,
                            w2[:, kc, mc * P:(mc + 1) * P],
                            hT[:, kc, :],
                            start=(kc == 0), stop=(kc == NKC - 1))
                    nc.vector.tensor_add(y_acc[:, mc, :], y_acc[:, mc, :],
                                         ps)

            for m in range(NKC):
                nc.vector.tensor_scalar_add(y_acc[:, m, :], y_acc[:, m, :],
                                            ffb2_sb[:, m:m + 1])

            # =============== final LN + output ==========================
            layer_norm(
                y_acc, "ln3g", "ln3b", None,
                lambda m: chunk.tile([P, 512], F32, tag="stream", bufs=3,
                                     name="out_ch"),
                lambda m, fc: nc.sync.dma_start(outT[:, m, :], fc))

    nc.finalize()
    return nc


# ---------------------------------------------------------------------------
# host side
# ---------------------------------------------------------------------------

def _wl(w):
    """[K, N] -> [128, K/128, N] bf16 (K on partitions, chunked)."""
    k, n = w.shape
    return np.ascontiguousarray(
        w.reshape(k // P, P, n).transpose(1, 0, 2)).astype(BF16)


def _tl(a):
    """[S, D] activation -> transposed layout [128, D/128, S]."""
    t = a.T  # [D, S]
    return np.ascontiguousarray(
        t.reshape(D // P, P, t.shape[1]).transpose(1, 0, 2))


def _vc(v, n):
    """[n*128] vector -> [128, n] (chunk layout)."""
    return np.ascontiguousarray(v.reshape(n, P).T)


def _binmask(tiles):
    m = np.zeros((P, NQT, 2, P), np.float32)
    tri = (np.arange(P)[:, None] <= np.arange(P)[None, :]).astype(np.float32)
    for l in range(NQT):
        g = tiles[l]
        for j in range(2):
            kt = NKMAX[l] - 2 + j
            if kt < g:
                m[:, l, j, :] = 1.0
            elif kt == g:
                m[:, l, j, :] = tri
            else:
                m[:, l, j, :] = 0.0
    return m.astype(BF16)


_NC_CACHE = None


def _get_nc():
    global _NC_CACHE
    if _NC_CACHE is None:
        _NC_CACHE = build_program()
    return _NC_CACHE


def make_in_maps(inputs):
    x = np.asarray(inputs["decoder_inputs"], np.float32)
    enc = np.asarray(inputs["encoder_output"], np.float32)
    pad = np.asarray(inputs["padding_mask"], np.float32)

    shared = {
        "w_sa_q": _wl(np.asarray(inputs["sa_wq"], np.float32)),
        "w_sa_k": _wl(np.asarray(inputs["sa_wk"], np.float32)),
        "w_sa_v": _wl(np.asarray(inputs["sa_wv"], np.float32)),
        "w_sa_o": _wl(np.asarray(inputs["sa_wo"], np.float32)),
        "w_ca_q": _wl(np.asarray(inputs["ca_wq"], np.float32)),
        "w_ca_k": _wl(np.asarray(inputs["ca_wk"], np.float32)),
        "w_ca_v": _wl(np.asarray(inputs["ca_wv"], np.float32)),
        "w_ca_o": _wl(np.asarray(inputs["ca_wo"], np.float32)),
    }
    ffw1 = np.asarray(inputs["ff_w1"], np.float32)
    ffw2 = np.asarray(inputs["ff_w2"], np.float32)
    for qi in range(4):
        shared[f"w_ff1_{qi}"] = _wl(ffw1[:, qi * 1024:(qi + 1) * 1024])
        shared[f"w_ff2_{qi}"] = _wl(ffw2[qi * 1024:(qi + 1) * 1024, :])
    for i in (1, 2, 3):
        shared[f"ln{i}g"] = _vc(np.asarray(inputs[f"ln{i}_g"], np.float32), NKC)
        shared[f"ln{i}b"] = _vc(np.asarray(inputs[f"ln{i}_b"], np.float32), NKC)
    shared["ffb1"] = _vc(np.asarray(inputs["ff_b1"], np.float32), 32)
    shared["ffb2"] = _vc(np.asarray(inputs["ff_b2"], np.float32), NKC)

    bm_even = _binmask(TILES_EVEN)
    bm_odd = _binmask(TILES_ODD)

    in_maps = []
    for c in range(8):
        b = c // 2
        tiles = TILES_EVEN if c % 2 == 0 else TILES_ODD
        qrows = np.concatenate(
            [np.arange(t * P, (t + 1) * P) for t in tiles])
        xb = x[b]
        m = dict(shared)
        m["xT"] = _tl(xb).astype(BF16)
        xq = xb[qrows]
        xqt = _tl(xq)
        m["xqT"] = xqt.astype(BF16)
        m["xqf"] = xqt.astype(np.float32)
        m["encT"] = _tl(enc[b]).astype(BF16)
        m["binmask"] = bm_even if c % 2 == 0 else bm_odd
        m["crossbias"] = _vc(pad[b, 0, 0, :] * np.float32(-1e9), NKC)
        in_maps.append(m)
    return in_maps


def assemble_output(results):
    out = np.empty((4, S, D), np.float32)
    for c in range(8):
        b = c // 2
        tiles = TILES_EVEN if c % 2 == 0 else TILES_ODD
        ot = np.asarray(results[c]["outT"], np.float32).reshape(P, NKC, Q)
        full_t = ot.transpose(1, 0, 2).reshape(D, Q)     # [d, qlocal]
        for l, t in enumerate(tiles):
            out[b, t * P:(t + 1) * P, :] = full_t[:, l * P:(l + 1) * P].T
    return out


def kernel(**inputs):
    from concourse.bass_utils import run_bass_kernel_spmd

    nc = _get_nc()
    in_maps = make_in_maps(inputs)
    res = run_bass_kernel_spmd(nc, in_maps, core_ids=list(range(8)))
    return assemble_output(res.results)
